# revision 8
# baseline (speedup 1.0000x reference)
"""Trainium2 Bass kernel for DualTierMiras (dual low-rank tier read + LayerNorm-gate mix).

Computes, for k [N, d]:
    v_t   = k @ (SCALE * tanh(B_t @ C_t.T) + diag(D_t)).T      (t in {fast, deep})
    h     = LayerNorm(k) * gamma + beta
    w     = sigmoid(silu(h @ W1.T + b1) @ W2.T + b2 + base_logit)
    out   = w * v_fast + (1 - w) * v_deep

Strategy: data-parallel over rows across 8 NeuronCores. All device matmuls
contract over d, so every tensor is kept in a transposed layout ([d, rows]):
the host passes k.T shards and W1.T, and the device returns out.T shards.

Device variants:
  * "lr8" (fast path): lowrank tiers + fp8 DoubleRow gate matmul with the
    LayerNorm folded OUT of the critical path:
        s1 = rstd * (k @ (gamma*W1).T - mu * rowsum(gamma*W1)) + b1'
    so the big K=2048 gate matmul runs directly on raw k (as fp8) and starts
    while k is still streaming in; the per-row (mu, rstd) correction is a
    2-op DVE epilogue on each PSUM tile. fp8 e4m3 + DoubleRow packs two
    K-rows per PE pass (2x matmul throughput) and halves the W1 DMA.
  * "lowrank": bf16 fallback. tanh(u) ~= u whenever max|u| is provably tiny;
    then the tier reads are rank-32 and both tiers fuse into one K=64 matmul.
  * "tanh": materializes tanh(C B^T) per 512-column block on device and does
    the full dense tier matmuls. Used when the linearization is not safe.

Tier reads/stats always run in bf16 (fp8 would put ~3% error directly on the
output); only the gate matmul (whose output passes through a sigmoid into a
0..1 mixing weight) tolerates fp8.
"""

from contextlib import ExitStack

import numpy as np

N, D, R = 8192, 2048, 32
NCORES = 8
NSH = N // NCORES          # rows per core
P = 128                    # SBUF partitions
NJ = D // P                # 16 chunks of d
FH = 512                   # free-dim half of NSH (PSUM bank width in fp32)
NH = NSH // FH             # 2 halves
SCALE = 0.1
LN_EPS = 1e-5
# max |B C^T| element below which tanh(u) ~= u is used (per-element relative
# error of the tanh factor <= thr^2/3 ~= 0.33%).
LOWRANK_THR = 0.10

_NC_CACHE: dict = {}


# ---------------------------------------------------------------- device build

def build_nc(mode: str, has_d: bool, repeat: int = 1, sim_safe: bool = False):
    import concourse.bacc as bacc
    import concourse.tile as tile
    from concourse import mybir

    f32 = mybir.dt.float32
    nc = bacc.Bacc("TRN2", target_bir_lowering=False, debug=False,
                   num_devices=NCORES)

    bf16 = mybir.dt.bfloat16
    if mode == "lr8":
        f8 = mybir.dt.float8e4
        kt_d = nc.dram_tensor("kt", [D, NSH], bf16, kind="ExternalInput")
        w18_d = nc.dram_tensor("w18", [P, NJ * D], f8, kind="ExternalInput")
        pv_d = nc.dram_tensor("pv", [P, 48], f32, kind="ExternalInput")
        sc_d = nc.dram_tensor("sc", [1, 1], f32, kind="ExternalInput")
        caug_d = nc.dram_tensor("caug", [D, 65], f32, kind="ExternalInput")
        bt_d = nc.dram_tensor("bt", [64, D], f32, kind="ExternalInput")
        out_d = nc.dram_tensor("outT", [D, NSH], bf16, kind="ExternalOutput")
        with tile.TileContext(nc) as tc:
            for _ in range(repeat):
                with ExitStack() as ctx:
                    _emit_lr8(ctx, tc, nc, kt_d, w18_d, pv_d, sc_d, caug_d,
                              bt_d, out_d, sim_safe=sim_safe)
        nc.compile()
        return nc
    kt_d = nc.dram_tensor("kt", [D, NSH], bf16, kind="ExternalInput")
    w1t_d = nc.dram_tensor("w1t", [D, D], bf16, kind="ExternalInput")
    pv_d = nc.dram_tensor("pv", [P, 64], f32, kind="ExternalInput")
    sc_d = nc.dram_tensor("sc", [1, 1], f32, kind="ExternalInput")
    bt_d = nc.dram_tensor("bt", [64, D], f32, kind="ExternalInput")
    caug_d = ct_d = dv_d = None
    if mode == "lowrank":
        caug_d = nc.dram_tensor("caug", [D, 65], f32, kind="ExternalInput")
    else:
        ct_d = nc.dram_tensor("ct", [64, D], f32, kind="ExternalInput")
    if has_d:
        dv_d = nc.dram_tensor("dv", [P, 32], f32, kind="ExternalInput")
    # bf16 output (inputs to every product term are bf16 already); the host
    # upcasts to f32 when unsharding. Halves the output DMA on the tail.
    out_d = nc.dram_tensor("outT", [D, NSH], bf16, kind="ExternalOutput")

    with tile.TileContext(nc) as tc:
        for _ in range(repeat):
            with ExitStack() as ctx:
                _emit(ctx, tc, nc, mode, has_d,
                      kt_d, w1t_d, pv_d, sc_d, bt_d, caug_d, ct_d, dv_d, out_d,
                      sim_safe=sim_safe)
    nc.compile()
    return nc


def _emit_lr8(ctx, tc, nc, kt_d, w18_d, pv_d, sc_d, caug_d, bt_d, out_d,
              sim_safe=False):
    """fp8 DoubleRow gate + lowrank tiers, LN as a PSUM epilogue.

    pv columns: 0..15  r    = rowsum(gamma*W1) chunks   (f32)
                16..31 b1'  = b1 + beta @ W1.T chunks
                32..47 W2 chunks
    w18 layout: [p, oc*NJ*P + jj*2*P + i*P + m] = (gamma*W1)[oc*P+m, (2jj+i)*P+p]
    """
    from concourse import mybir

    f32 = mybir.dt.float32
    bf16 = mybir.dt.bfloat16
    f8 = mybir.dt.float8e4
    AF = mybir.ActivationFunctionType
    ALU = mybir.AluOpType
    DR = mybir.MatmulPerfMode.DoubleRow
    NJJ = NJ // 2

    const = ctx.enter_context(tc.tile_pool(name="const", bufs=1))
    persist = ctx.enter_context(tc.tile_pool(name="persist", bufs=1))
    w1p = ctx.enter_context(tc.tile_pool(name="w1p", bufs=1))
    tmp = ctx.enter_context(tc.tile_pool(name="tmp", bufs=3))
    h2pool = ctx.enter_context(tc.tile_pool(name="h2p", bufs=2))
    outpool = ctx.enter_context(tc.tile_pool(name="outp", bufs=3))
    small = ctx.enter_context(tc.tile_pool(name="small", bufs=1))
    svec = ctx.enter_context(tc.tile_pool(name="svec", bufs=4))
    sqpool = ctx.enter_context(tc.tile_pool(name="sqp", bufs=2))
    psBC = ctx.enter_context(tc.tile_pool(name="psBC", bufs=1, space="PSUM"))

    # ---- constants ------------------------------------------------------
    pv = const.tile([P, 48], f32, tag="pv", name="pv")
    nc.sync.dma_start(pv[:], pv_d[:])
    sc = const.tile([1, 1], f32, tag="sc", name="sc")
    nc.sync.dma_start(sc[:], sc_d[:])
    caug_v = caug_d[:].rearrange("(j p) r -> p j r", p=P)
    with tc.tile_pool(name="caugstage", bufs=1) as caugstage:
        cf3 = caugstage.tile([P, NJ, 65], f32, tag="caugf", name="caugf")
        nc.sync.dma_start(cf3[:], caug_v[:])
        caug_bf = const.tile([P, NJ, 65], bf16, tag="caugbf", name="caugbf")
        nc.vector.tensor_copy(caug_bf[:], cf3[:])
    ones_row = const.tile([1, P], bf16, tag="onesrow", name="onesrow")
    nc.vector.memset(ones_row[:], 1.0)
    neg_row = const.tile([1, P], bf16, tag="negrow", name="negrow")
    nc.vector.memset(neg_row[:], -1.0)
    # [P, 2, 16] (not [P, 2, 1]): dual-fp8 ldweights requires the outer
    # free-AP step to be 16B-aligned, so the pair stride must be >= 16.
    ones8 = const.tile([P, 2, 16], f8, tag="ones8", name="ones8")
    nc.vector.memset(ones8[:], 1.0)
    w2_bf = const.tile([P, NJ], bf16, tag="w2bf", name="w2bf")
    nc.vector.tensor_copy(w2_bf[:], pv[:, 32:48])

    # ---- resident k tiles + phase A (stats) while k streams -------------
    kt_bf = persist.tile([P, NJ, NSH], bf16, tag="ktbf", name="kt_bf")
    kt8 = persist.tile([P, NJ, NSH], f8, tag="kt8", name="kt8")
    w1sb = [w1p.tile([P, NJJ, 2, P], f8, tag=f"w1_{oc}", name=f"w1_{oc}")
            for oc in range(NJ)]
    mu = [small.tile([1, FH], f32, tag=f"mu{h}", name=f"mu{h}") for h in range(NH)]
    msq = [small.tile([1, FH], f32, tag=f"msq{h}", name=f"msq{h}") for h in range(NH)]
    G_sb = [persist.tile([64, FH], bf16, tag=f"gsb{h}", name=f"gsb{h}")
            for h in range(NH)]
    mu_b = [persist.tile([P, FH], bf16, tag=f"mub{h}", name=f"mub{h}")
            for h in range(NH)]
    nrstd_b = [persist.tile([P, FH], bf16, tag=f"nrstdb{h}", name=f"nrstdb{h}")
               for h in range(NH)]
    wv = [persist.tile([1, FH], f32, tag=f"wv{h}", name=f"wv{h}")
          for h in range(NH)]

    # DMA order: first two W1 chunks early so gate col 0/1 never stall, then
    # all of k (the critical stream), then the rest of W1, then tier factors.
    nc.sync.dma_start(w1sb[0][:], w18_d[:, 0:D])
    nc.sync.dma_start(w1sb[1][:], w18_d[:, D:2 * D])

    with tc.tile_pool(name="psA", bufs=1, space="PSUM") as psA:
        psum_G = [psA.tile([65, FH], f32, tag=f"psG{h}", name=f"psG{h}")
                  for h in range(NH)]
        psum_Q = [psA.tile([16, FH], f32, tag=f"psQ{h}", name=f"psQ{h}")
                  for h in range(NH)]
        for jj in range(NJJ):
            for i in range(2):
                j = 2 * jj + i
                nc.sync.dma_start(kt_bf[:, j, :], kt_d[j * P:(j + 1) * P, :])
                nc.vector.tensor_copy(kt8[:, j, :], kt_bf[:, j, :])
                for h in range(NH):
                    sl = slice(h * FH, (h + 1) * FH)
                    nc.tensor.matmul(psum_G[h][:], caug_bf[:, j, :],
                                     kt_bf[:, j, sl],
                                     start=(j == 0), stop=(j == NJ - 1))
            sq = sqpool.tile([P, 2, NSH], f8, tag="sq", name="sq")
            for i in range(2):
                j = 2 * jj + i
                nc.vector.tensor_mul(sq[:, i, :], kt_bf[:, j, :], kt_bf[:, j, :])
            for h in range(NH):
                sl = slice(h * FH, (h + 1) * FH)
                nc.tensor.matmul(psum_Q[h][:], ones8[:], sq[:, :, sl],
                                 start=(jj == 0), stop=(jj == NJJ - 1),
                                 perf_mode=DR)

        for oc in range(2, NJ):
            nc.sync.dma_start(w1sb[oc][:], w18_d[:, oc * D:(oc + 1) * D])
        bt_bf = const.tile([64, D], bf16, tag="btbf", name="btbf")
        with tc.tile_pool(name="facstage", bufs=2) as facstage:
            for q in range(2):
                qs = slice(q * (D // 2), (q + 1) * (D // 2))
                btf = facstage.tile([64, D // 2], f32, tag="btf", name="btf")
                nc.sync.dma_start(btf[:], bt_d[:, qs])
                nc.vector.tensor_copy(bt_bf[:, qs], btf[:])

        for h in range(NH):
            nc.scalar.mul(G_sb[h][:], psum_G[h][0:64, :], SCALE)
            nc.scalar.mul(mu[h][:], psum_G[h][64:65, :], 1.0 / D)
            nc.scalar.mul(msq[h][:], psum_Q[h][0:1, :], 1.0 / D)

    # ---- LN stat broadcasts (negated rstd so the epilogue is 2 DVE ops) -
    for h in range(NH):
        mu2 = svec.tile([1, FH], f32, tag="sv", name="mu2")
        nc.vector.tensor_mul(mu2[:], mu[h][:], mu[h][:])
        veps = svec.tile([1, FH], f32, tag="sv", name="veps")
        nc.vector.scalar_tensor_tensor(veps[:], msq[h][:], LN_EPS, mu2[:],
                                       op0=ALU.add, op1=ALU.subtract)
        rinv = svec.tile([1, FH], f32, tag="sv", name="rinv")
        nc.vector.reciprocal(rinv[:], veps[:])
        rstd_bf = svec.tile([1, FH], bf16, tag="sv", name="rstd_bf")
        nc.scalar.activation(rstd_bf[:], rinv[:], AF.Sqrt)
        mu_bf = svec.tile([1, FH], bf16, tag="sv", name="mu_bf")
        nc.scalar.copy(mu_bf[:], mu[h][:])
        pbm = psBC.tile([P, FH], f32, tag="pbc", name="pbm")
        nc.tensor.matmul(pbm[:], ones_row[0:1, :], mu_bf[:],
                         start=True, stop=True)
        nc.scalar.copy(mu_b[h][:], pbm[:])
        pbr = psBC.tile([P, FH], f32, tag="pbc", name="pbr")
        nc.tensor.matmul(pbr[:], neg_row[0:1, :], rstd_bf[:],
                         start=True, stop=True)
        nc.scalar.copy(nrstd_b[h][:], pbr[:])

    # ---- gate (fp8 DoubleRow) + epilogue + logit + tier -----------------
    def emit_silu(s1b, oc, h2):
        if sim_safe:
            sbt = h2pool.tile([P, FH], f32, tag="sb", name="sb")
            nc.scalar.activation(sbt[:], s1b[:], AF.Identity,
                                 bias=pv[:, 16 + oc:17 + oc])
            sig = h2pool.tile([P, FH], f32, tag="sig", name="sig")
            nc.scalar.activation(sig[:], s1b[:], AF.Sigmoid,
                                 bias=pv[:, 16 + oc:17 + oc])
            nc.vector.tensor_mul(h2[:], sbt[:], sig[:])
        else:
            nc.scalar.activation(h2[:], s1b[:], AF.Silu,
                                 bias=pv[:, 16 + oc:17 + oc])

    def emit_tier(h, psC):
        wv_bf = svec.tile([1, FH], bf16, tag="sv", name="wv_bf")
        nc.vector.tensor_copy(wv_bf[:], wv[h][:])
        pw = psC.tile([64, FH], f32, tag="vt", name="pw")
        nc.tensor.matmul(pw[:], ones_row[0:1, 0:64], wv_bf[:],
                         start=True, stop=True)
        wcat = persist.tile([64, FH], bf16, tag=f"wcat{h}", name=f"wcat{h}")
        nc.scalar.copy(wcat[0:32, :], pw[0:32, :])
        nc.scalar.activation(wcat[32:64, :], pw[32:64, :], AF.Copy,
                             bias=1.0, scale=-1.0)
        Gw = persist.tile([64, FH], bf16, tag=f"gw{h}", name=f"gw{h}")
        nc.vector.tensor_mul(Gw[:], G_sb[h][:], wcat[:])
        for m in range(NJ):
            pvt = psC.tile([P, FH], f32, tag="vt", name="vt")
            nc.tensor.matmul(pvt[:], bt_bf[0:64, m * P:(m + 1) * P],
                             Gw[:], start=True, stop=True)
            ot = outpool.tile([P, FH], bf16, tag="ot", name="ot")
            if m % 2 == 0:
                nc.scalar.copy(ot[:], pvt[:])
            else:
                nc.vector.tensor_copy(ot[:], pvt[:])
            nc.sync.dma_start(
                out_d[m * P:(m + 1) * P, h * FH:(h + 1) * FH], ot[:])

    with tc.tile_pool(name="psB", bufs=3, space="PSUM") as psB, \
         tc.tile_pool(name="psL", bufs=1, space="PSUM") as psL, \
         tc.tile_pool(name="psC", bufs=2, space="PSUM") as psC:
        psum_L = [psL.tile([1, FH], f32, tag=f"psL{h}", name=f"psL{h}")
                  for h in range(NH)]
        for h in range(NH):
            sl = slice(h * FH, (h + 1) * FH)
            for oc in range(NJ):
                s1p = psB.tile([P, FH], f32, tag="s1", name="s1")
                for jj in range(NJJ):
                    nc.tensor.matmul(s1p[:], w1sb[oc][:, jj, :, :],
                                     kt8[:, 2 * jj:2 * jj + 2, sl],
                                     start=(jj == 0), stop=(jj == NJJ - 1),
                                     perf_mode=DR)
                t1 = tmp.tile([P, FH], bf16, tag="ep1", name="ep1")
                nc.vector.scalar_tensor_tensor(t1[:], mu_b[h][:],
                                               pv[:, oc:oc + 1], s1p[:],
                                               op0=ALU.mult, op1=ALU.subtract)
                s1b = tmp.tile([P, FH], bf16, tag="ep2", name="s1b")
                nc.vector.tensor_mul(s1b[:], t1[:], nrstd_b[h][:])
                h2 = h2pool.tile([P, FH], bf16, tag="h2", name="h2")
                emit_silu(s1b, oc, h2)
                nc.tensor.matmul(psum_L[h][:], w2_bf[:, oc:oc + 1], h2[:],
                                 start=(oc == 0), stop=(oc == NJ - 1))
            nc.scalar.activation(wv[h][:], psum_L[h][:], AF.Sigmoid,
                                 bias=sc[0:1, 0:1])
            emit_tier(h, psC)


def _emit(ctx, tc, nc, mode, has_d,
          kt_d, w1t_d, pv_d, sc_d, bt_d, caug_d, ct_d, dv_d, out_d,
          sim_safe=False):
    import concourse.bass as bass  # noqa: F401
    from concourse import mybir

    f32 = mybir.dt.float32
    bf16 = mybir.dt.bfloat16
    AF = mybir.ActivationFunctionType
    ALU = mybir.AluOpType
    lowrank = mode == "lowrank"

    const = ctx.enter_context(tc.tile_pool(name="const", bufs=1))
    persist = ctx.enter_context(tc.tile_pool(name="persist", bufs=1))
    stage = ctx.enter_context(tc.tile_pool(name="stage", bufs=2))
    tmp = ctx.enter_context(tc.tile_pool(name="tmp", bufs=3))
    kt2pool = ctx.enter_context(tc.tile_pool(name="kt2p", bufs=2))
    h2pool = ctx.enter_context(tc.tile_pool(name="h2p", bufs=2))
    outpool = ctx.enter_context(tc.tile_pool(name="outp", bufs=3))
    small = ctx.enter_context(tc.tile_pool(name="small", bufs=1))
    # rotating slots for short-lived [1, FH] vectors (each costs a full
    # free-dim slot across all partitions, so don't give each a unique tag)
    svec = ctx.enter_context(tc.tile_pool(name="svec", bufs=4))

    # ---- small constants -------------------------------------------------
    pv = const.tile([P, 64], f32, tag="pv", name="pv")
    nc.sync.dma_start(pv[:], pv_d[:])
    sc = const.tile([1, 1], f32, tag="sc", name="sc")
    nc.sync.dma_start(sc[:], sc_d[:])

    if lowrank:
        # one 3D-AP DMA + one cast instead of 16 tiny strided loads, so the
        # DMA queue reaches the kt tiles sooner
        caug_v = caug_d[:].rearrange("(j p) r -> p j r", p=P)
        with tc.tile_pool(name="caugstage", bufs=1) as caugstage:
            cf3 = caugstage.tile([P, NJ, 65], f32, tag="caugf", name="caugf")
            nc.sync.dma_start(cf3[:], caug_v[:])
            cb3 = const.tile([P, NJ, 65], bf16, tag="caugbf", name="caugbf")
            nc.vector.tensor_copy(cb3[:], cf3[:])
        caug_bf = [cb3[:, j, :] for j in range(NJ)]
    ones_col = const.tile([P, 1], bf16, tag="ones", name="ones")
    nc.vector.memset(ones_col[:], 1.0)
    ones_row = const.tile([1, P], bf16, tag="onesrow", name="onesrow")
    nc.vector.memset(ones_row[:], 1.0)
    # gpsimd.partition_broadcast writes garbage on HW via this compile path;
    # broadcast [1, FH] rows across partitions with a K=1 matmul instead.
    psBC = ctx.enter_context(tc.tile_pool(name="psBC", bufs=1, space="PSUM"))

    def bcast_psum(src_row_bf16, nparts):
        pb = psBC.tile([nparts, FH], f32, tag="pbc", name="pbc")
        nc.tensor.matmul(pb[:], ones_row[0:1, 0:nparts], src_row_bf16[:],
                         start=True, stop=True)
        return pb

    w2_bf = const.tile([P, NJ], bf16, tag="w2bf", name="w2bf")
    nc.vector.tensor_copy(w2_bf[:], pv[:, 48:64])

    if has_d:
        dv = const.tile([P, 32], f32, tag="dv", name="dv")
        nc.sync.dma_start(dv[:], dv_d[:])

    # ---- load k.T, cast bf16, phase-A matmuls (stats + G) ----------------
    # kt_bf is dead after hT is built (unless a tier path needs it later), so
    # it lives in its own pool that the caller scopes appropriately.
    import os
    resident_gate = (lowrank and not has_d
                     and not os.environ.get('K_NO_RESIDENT'))
    htpool = ctx.enter_context(tc.tile_pool(name="htp", bufs=1))
    ktpool = persist
    if resident_gate:
        # resident bf16 W1 j-tiles, prefetched during the prologue
        w1pool = ctx.enter_context(tc.tile_pool(name="w1p", bufs=1))
    kt_bf = [ktpool.tile([P, NSH], bf16, tag=f"ktbf{j}", name=f"ktbf{j}") for j in range(NJ)]
    mu = [small.tile([1, FH], f32, tag=f"mu{h}", name=f"mu{h}") for h in range(NH)]
    msq = [small.tile([1, FH], f32, tag=f"msq{h}", name=f"msq{h}") for h in range(NH)]
    G_sb = None
    if lowrank:
        G_sb = [persist.tile([64, FH], bf16, tag=f"gsb{h}", name=f"gsb{h}") for h in range(NH)]

    with tc.tile_pool(name="psA", bufs=1, space="PSUM") as psA:
        if lowrank:
            psum_G = [psA.tile([65, FH], f32, tag=f"psG{h}", name=f"psG{h}") for h in range(NH)]
        else:
            psum_S = [psA.tile([1, FH], f32, tag=f"psS{h}", name=f"psS{h}") for h in range(NH)]
        psum_Q = [psA.tile([1, FH], f32, tag=f"psQ{h}", name=f"psQ{h}") for h in range(NH)]

        for j in range(NJ):
            st, sp = j == 0, j == NJ - 1
            nc.sync.dma_start(kt_bf[j][:], kt_d[j * P:(j + 1) * P, :])
            kt2 = kt2pool.tile([P, NSH], bf16, tag="kt2", name="kt2")
            nc.vector.tensor_mul(kt2[:], kt_bf[j][:], kt_bf[j][:])
            for h in range(NH):
                sl = slice(h * FH, (h + 1) * FH)
                if lowrank:
                    nc.tensor.matmul(psum_G[h][:], caug_bf[j][:],
                                     kt_bf[j][:, sl], start=st, stop=sp)
                else:
                    nc.tensor.matmul(psum_S[h][:], ones_col[:],
                                     kt_bf[j][:, sl], start=st, stop=sp)
                nc.tensor.matmul(psum_Q[h][:], ones_col[:],
                                 kt2[:, sl], start=st, stop=sp)

        w1o = None
        if resident_gate:
            # per-o column blocks via 3D AP: arrival order == the gate's
            # consumption order, so o=0 can start after 1 MB instead of 8 MB.
            w1v = w1t_d[:].rearrange("(j p) o -> p j o", p=P)
            w1o = [w1pool.tile([P, NJ, P], bf16, tag=f"w1o{o}", name=f"w1o{o}")
                   for o in range(NJ)]
            for o in range(NJ):
                nc.sync.dma_start(w1o[o][:], w1v[:, :, o * P:(o + 1) * P])

        for h in range(NH):
            if lowrank:
                nc.scalar.mul(G_sb[h][:], psum_G[h][0:64, :], SCALE)
                nc.scalar.mul(mu[h][:], psum_G[h][64:65, :], 1.0 / D)
            else:
                nc.scalar.mul(mu[h][:], psum_S[h][:], 1.0 / D)
            nc.scalar.mul(msq[h][:], psum_Q[h][:], 1.0 / D)

    # factor tiles (used only by the tier reads, so loaded after kt+W1 to
    # keep them off the critical DMA prologue): joint [64, D] for the lowrank
    # K=64 fused matmul; split per-tier [32, D] tiles in tanh mode.
    if lowrank:
        bt_bf = const.tile([64, D], bf16, tag="btbf", name="btbf")
    else:
        btt_bf = [const.tile([32, D], bf16, tag=f"btbf{t}", name=f"btbf{t}")
                  for t in range(2)]
        ctt_bf = [const.tile([32, D], bf16, tag=f"ctbf{t}", name=f"ctbf{t}")
                  for t in range(2)]
    with tc.tile_pool(name="facstage", bufs=2) as facstage:
        for q in range(2):
            qs = slice(q * (D // 2), (q + 1) * (D // 2))
            if lowrank:
                btf = facstage.tile([64, D // 2], f32, tag="btf", name="btf")
                nc.sync.dma_start(btf[:], bt_d[:, qs])
                nc.vector.tensor_copy(bt_bf[:, qs], btf[:])
            else:
                for t in range(2):
                    btf = facstage.tile([32, D // 2], f32, tag="btf", name="btf")
                    nc.sync.dma_start(btf[:], bt_d[32 * t:32 * t + 32, qs])
                    nc.vector.tensor_copy(btt_bf[t][:, qs], btf[:])
                    ctf = facstage.tile([32, D // 2], f32, tag="ctf", name="ctf")
                    nc.sync.dma_start(ctf[:], ct_d[32 * t:32 * t + 32, qs])
                    nc.vector.tensor_copy(ctt_bf[t][:, qs], ctf[:])

    # ---- LN stats finalize + broadcast -----------------------------------
    mu_b = [persist.tile([P, FH], bf16, tag=f"mub{h}", name=f"mub{h}") for h in range(NH)]
    rstd_b = [persist.tile([P, FH], bf16, tag=f"rstdb{h}", name=f"rstdb{h}") for h in range(NH)]
    for h in range(NH):
        mu2 = svec.tile([1, FH], f32, tag="sv", name="mu2")
        nc.vector.tensor_mul(mu2[:], mu[h][:], mu[h][:])
        veps = svec.tile([1, FH], f32, tag="sv", name="veps")
        # (msq + eps) - mu^2
        nc.vector.scalar_tensor_tensor(veps[:], msq[h][:], LN_EPS, mu2[:],
                                       op0=ALU.add, op1=ALU.subtract)
        rinv = svec.tile([1, FH], f32, tag="sv", name="rinv")
        nc.vector.reciprocal(rinv[:], veps[:])
        rstd_bf = svec.tile([1, FH], bf16, tag="sv", name="rstd_bf")
        nc.scalar.activation(rstd_bf[:], rinv[:], AF.Sqrt)
        mu_bf = svec.tile([1, FH], bf16, tag="sv", name="mu_bf")
        nc.scalar.copy(mu_bf[:], mu[h][:])
        nc.scalar.copy(mu_b[h][:], bcast_psum(mu_bf, P)[:])
        nc.scalar.copy(rstd_b[h][:], bcast_psum(rstd_bf, P)[:])

    # ---- gate: h = LN(k)*gamma+beta; silu(h @ W1.T + b1); logit ----------
    wv = [svec.tile([1, FH], f32, tag="wvlong", bufs=2, name=f"wv{h}")
          for h in range(NH)]
    # W1.T viewed as [p, j, o] so one DMA fetches the [2048, 128] column block
    # for a given o-chunk as a [128, 16, 128] tile (partition dim = j rows).
    w1t_v = w1t_d[:].rearrange("(j p) o -> p j o", p=P)

    def emit_ln():
        ht = [htpool.tile([P, NSH], bf16, tag=f"ht{j}", name=f"ht{j}")
              for j in range(NJ)]
        for h in range(NH):           # h-major: h=0 tiles finish first
            for j in range(NJ):
                sl = slice(h * FH, (h + 1) * FH)
                t1 = tmp.tile([P, FH], bf16, tag="lnt1", name="lnt1")
                nc.vector.tensor_sub(t1[:], kt_bf[j][:, sl], mu_b[h][:])
                t2 = tmp.tile([P, FH], bf16, tag="lnt2", name="lnt2")
                nc.vector.tensor_mul(t2[:], t1[:], rstd_b[h][:])
                nc.scalar.activation(ht[j][:, sl], t2[:], AF.Identity,
                                     bias=pv[:, 16 + j:17 + j],
                                     scale=pv[:, j:j + 1])
        return ht

    def emit_silu(s1, o, h2):
        if sim_safe:
            # CoreSim has no Silu LUT; decompose (sim-only build).
            sbt = h2pool.tile([P, FH], f32, tag="sb", name="sb")
            nc.scalar.activation(sbt[:], s1[:], AF.Identity,
                                 bias=pv[:, 32 + o:33 + o])
            sig = h2pool.tile([P, FH], f32, tag="sig", name="sig")
            nc.scalar.activation(sig[:], s1[:], AF.Sigmoid,
                                 bias=pv[:, 32 + o:33 + o])
            nc.vector.tensor_mul(h2[:], sbt[:], sig[:])
        else:
            nc.scalar.activation(h2[:], s1[:], AF.Silu,
                                 bias=pv[:, 32 + o:33 + o])

    def emit_gate_col(psB, psum_L, w1b_j_aps, o, h):
        """One (o, h) gate column: 16 accumulating matmuls + silu + logit."""
        sl = slice(h * FH, (h + 1) * FH)
        s1 = psB.tile([P, FH], f32, tag="s1", name="s1")
        for j in range(NJ):
            nc.tensor.matmul(s1[:], w1b_j_aps[j], ht[j][:, sl],
                             start=(j == 0), stop=(j == NJ - 1))
        h2 = h2pool.tile([P, FH], bf16, tag="h2", name="h2")
        emit_silu(s1, o, h2)
        nc.tensor.matmul(psum_L[h][:], w2_bf[:, o:o + 1], h2[:],
                         start=(o == 0), stop=(o == NJ - 1))

    def emit_tier_lowrank(h, psC):
        """w -> Gw -> fused K=64 tier matmul -> out DMA, for one n-half."""
        wv_bf = svec.tile([1, FH], bf16, tag="sv", name="wv_bf")
        nc.vector.tensor_copy(wv_bf[:], wv[h][:])
        nb = P if has_d else 64
        pw = bcast_psum(wv_bf, nb)
        wcat = persist.tile([64, FH], bf16, tag=f"wcat{h}", name=f"wcat{h}")
        nc.scalar.copy(wcat[0:32, :], pw[0:32, :])
        nc.scalar.activation(wcat[32:64, :], pw[32:64, :], AF.Copy,
                             bias=1.0, scale=-1.0)
        if has_d:
            wb = persist.tile([P, FH], bf16, tag=f"wb128{h}", name=f"wb128{h}")
            nc.scalar.copy(wb[:], pw[:])
        Gw = persist.tile([64, FH], bf16, tag=f"gw{h}", name=f"gw{h}")
        nc.vector.tensor_mul(Gw[:], G_sb[h][:], wcat[:])
        for m in range(NJ):
            pvt = psC.tile([P, FH], f32, tag="vt", name="vt")
            nc.tensor.matmul(pvt[:], bt_bf[0:64, m * P:(m + 1) * P],
                             Gw[:], start=True, stop=True)
            ot = outpool.tile([P, FH], bf16, tag="ot", name="ot")
            if not has_d:
                # alternate engines so the psum->sbuf copies don't pile up
                if m % 2 == 0:
                    nc.scalar.copy(ot[:], pvt[:])
                else:
                    nc.vector.tensor_copy(ot[:], pvt[:])
            else:
                sl = slice(h * FH, (h + 1) * FH)
                dmix = tmp.tile([P, FH], bf16, tag="dmix", name="dmix")
                nc.vector.tensor_scalar(dmix[:], wb[:],
                                        dv[:, m:m + 1], dv[:, 16 + m:17 + m],
                                        op0=ALU.mult, op1=ALU.add)
                c = tmp.tile([P, FH], f32, tag="dc", name="dc")
                nc.vector.tensor_mul(c[:], kt_bf[m][:, sl], dmix[:])
                nc.vector.tensor_add(ot[:], pvt[:], c[:])
            nc.sync.dma_start(
                out_d[m * P:(m + 1) * P, h * FH:(h + 1) * FH], ot[:])

    if resident_gate:
        # Resident bf16 W1: one DMA+cast pass, reused by both n-halves, so
        # the gate runs h-outer and half 0's tier-read/output tail overlaps
        # half 1's gate matmuls.  kt_bf's pool closes once hT exists.
        ht = emit_ln()
        with ExitStack() as gctx:
            psC = gctx.enter_context(tc.tile_pool(name="psC", bufs=2,
                                                  space="PSUM"))
            with tc.tile_pool(name="psB", bufs=2, space="PSUM") as psB, \
                 tc.tile_pool(name="psL", bufs=1, space="PSUM") as psL:
                psum_L = [psL.tile([1, FH], f32, tag=f"psL{h}",
                                   name=f"psL{h}") for h in range(NH)]
                interleave = not os.environ.get('K_NO_INTERLEAVE')
                for h in range(NH):
                    for o in range(NJ):
                        aps = [w1o[o][:, j, :] for j in range(NJ)]
                        emit_gate_col(psB, psum_L, aps, o, h)
                    nc.scalar.activation(wv[h][:], psum_L[h][:], AF.Sigmoid,
                                         bias=sc[0:1, 0:1])
                    if interleave:
                        emit_tier_lowrank(h, psC)
                if not interleave:
                    for h in range(NH):
                        emit_tier_lowrank(h, psC)
    else:
        ht = emit_ln()
        with ExitStack() as gctx:
            w1bp = gctx.enter_context(tc.tile_pool(name="w1bp", bufs=2))
            with tc.tile_pool(name="psB", bufs=2, space="PSUM") as psB, \
                 tc.tile_pool(name="psL", bufs=1, space="PSUM") as psL:
                psum_L = [psL.tile([1, FH], f32, tag=f"psL{h}",
                                   name=f"psL{h}") for h in range(NH)]
                for o in range(NJ):
                    w1b = w1bp.tile([P, NJ, P], bf16, tag="w1b", name="w1b")
                    nc.sync.dma_start(w1b[:], w1t_v[:, :, o * P:(o + 1) * P])
                    for h in range(NH):
                        aps = [w1b[:, j, :] for j in range(NJ)]
                        emit_gate_col(psB, psum_L, aps, o, h)
                for h in range(NH):
                    nc.scalar.activation(wv[h][:], psum_L[h][:], AF.Sigmoid,
                                         bias=sc[0:1, 0:1])

    # ---- tier reads + mix ------------------------------------------------
    if lowrank:
        if not resident_gate:
            with tc.tile_pool(name="psC", bufs=3, space="PSUM") as psC:
                for h in range(NH):
                    emit_tier_lowrank(h, psC)
    else:
        # Full path: materialize M_t = tanh(C_t B_t^T) per 512-col block.
        wpb = [persist.tile([P, FH], f32, tag=f"wpb{h}", name=f"wpb{h}") for h in range(NH)]
        wqb = [persist.tile([P, FH], f32, tag=f"wqb{h}", name=f"wqb{h}") for h in range(NH)]
        wb128 = []
        for h in range(NH):
            wv_bf = svec.tile([1, FH], bf16, tag="sv", name="wv_bf")
            nc.vector.tensor_copy(wv_bf[:], wv[h][:])
            pw = bcast_psum(wv_bf, P)
            # wpb = SCALE*w, wqb = SCALE*(1-w), folded into the psum copies
            nc.scalar.mul(wpb[h][:], pw[:], SCALE)
            nc.scalar.activation(wqb[h][:], pw[:], AF.Copy,
                                 bias=SCALE, scale=-SCALE)
            if has_d:
                wb = persist.tile([P, FH], bf16, tag=f"wb128{h}", name=f"wb128{h}")
                nc.scalar.copy(wb[:], pw[:])
                wb128.append(wb)

        with ExitStack() as tctx:
            mpool = tctx.enter_context(tc.tile_pool(name="mtiles", bufs=1))
            psD = tctx.enter_context(tc.tile_pool(name="psD", bufs=2,
                                                  space="PSUM"))
            for mg in range(D // FH):
                mt = [[], []]
                for t in range(2):
                    for j in range(NJ):
                        pm = psD.tile([P, FH], f32, tag="pm", name="pm",
                                      bufs=1)
                        nc.tensor.matmul(
                            pm[:], ctt_bf[t][:, j * P:(j + 1) * P],
                            btt_bf[t][:, mg * FH:(mg + 1) * FH],
                            start=True, stop=True)
                        mtile = mpool.tile([P, FH], bf16, tag=f"m{t}_{j}", name=f"m{t}_{j}")
                        nc.scalar.activation(mtile[:], pm[:], AF.Tanh)
                        mt[t].append(mtile)
                for s in range(FH // P):
                    m = mg * (FH // P) + s
                    for h in range(NH):
                        sl = slice(h * FH, (h + 1) * FH)
                        pf = psD.tile([P, FH], f32, tag="pf", name="pf")
                        for j in range(NJ):
                            nc.tensor.matmul(pf[:],
                                             mt[0][j][:, s * P:(s + 1) * P],
                                             kt_bf[j][:, sl],
                                             start=(j == 0), stop=(j == NJ - 1))
                        pd_ = psD.tile([P, FH], f32, tag="pd", name="pd")
                        for j in range(NJ):
                            nc.tensor.matmul(pd_[:],
                                             mt[1][j][:, s * P:(s + 1) * P],
                                             kt_bf[j][:, sl],
                                             start=(j == 0), stop=(j == NJ - 1))
                        t0 = tmp.tile([P, FH], f32, tag="t0", name="t0")
                        nc.vector.tensor_mul(t0[:], pf[:], wpb[h][:])
                        t1 = tmp.tile([P, FH], f32, tag="t1", name="t1")
                        nc.vector.tensor_mul(t1[:], pd_[:], wqb[h][:])
                        ot = outpool.tile([P, FH], bf16, tag="ot", name="ot")
                        nc.vector.tensor_add(ot[:], t0[:], t1[:])
                        if has_d:
                            dmix = tmp.tile([P, FH], bf16, tag="dmix", name="dmix")
                            nc.vector.tensor_scalar(dmix[:], wb128[h][:],
                                                    dv[:, m:m + 1],
                                                    dv[:, 16 + m:17 + m],
                                                    op0=ALU.mult, op1=ALU.add)
                            c = tmp.tile([P, FH], f32, tag="dc", name="dc")
                            nc.vector.tensor_mul(c[:], kt_bf[m][:, sl], dmix[:])
                            ot2 = outpool.tile([P, FH], bf16, tag="ot2",
                                               name="ot2")
                            nc.vector.tensor_add(ot2[:], ot[:], c[:])
                            ot = ot2
                        nc.sync.dma_start(
                            out_d[m * P:(m + 1) * P, h * FH:(h + 1) * FH],
                            ot[:])


# ---------------------------------------------------------------- host side

def _chunked(vec):
    """[2048] -> [128, 16]; column j holds elements j*128 .. j*128+127."""
    return np.ascontiguousarray(np.asarray(vec, np.float32).reshape(NJ, P).T)


def _pick_mode(fast_B, fast_C, deep_B, deep_C):
    """lowrank iff max |B C^T| provably <= LOWRANK_THR."""
    worst = 0.0
    for B, C in ((fast_B, fast_C), (deep_B, deep_C)):
        bound = (np.linalg.norm(B, axis=1).max() *
                 np.linalg.norm(C, axis=1).max())
        if bound > LOWRANK_THR:
            bound = float(np.abs(B @ C.T).max())
        worst = max(worst, float(bound))
    return "lowrank" if worst <= LOWRANK_THR else "tanh"


def prepare(inputs):
    """-> (mode, has_d, in_maps) for the 8 cores."""
    import os
    g = {k: np.asarray(v, np.float32) for k, v in inputs.items()}
    k = g["k"]
    assert k.shape == (N, D), k.shape

    mode = _pick_mode(g["fast_B"], g["fast_C"], g["deep_B"], g["deep_C"])
    has_d = bool(np.any(g["fast_D"]) or np.any(g["deep_D"]))

    if mode == "lowrank" and not has_d and not os.environ.get("K_NO_FP8"):
        import ml_dtypes
        bf = ml_dtypes.bfloat16
        W = g["gate_W1"] * g["ln_gamma"][None, :]
        arr = W.reshape(NJ, P, NJ, P).transpose(3, 0, 2, 1)
        w18 = np.ascontiguousarray(arr.reshape(P, NJ * D)).astype(
            ml_dtypes.float8_e4m3)
        r = W.sum(axis=1)
        b1p = g["gate_b1"] + g["gate_W1"] @ g["ln_beta"]
        pv = np.concatenate([_chunked(r), _chunked(b1p),
                             _chunked(g["gate_W2"][0])], axis=1)
        common = {
            "w18": w18,
            "pv": pv,
            "sc": np.array([[g["gate_b2"][0] + g["base_logit"][0]]],
                           np.float32),
            "caug": np.ascontiguousarray(
                np.concatenate([g["fast_C"], g["deep_C"],
                                np.ones((D, 1), np.float32)], axis=1)),
            "bt": np.ascontiguousarray(
                np.concatenate([g["fast_B"].T, g["deep_B"].T], axis=0)),
        }
        in_maps = []
        for i in range(NCORES):
            m = dict(common)
            m["kt"] = np.ascontiguousarray(
                k[i * NSH:(i + 1) * NSH, :].T).astype(bf)
            in_maps.append(m)
        return "lr8", has_d, in_maps

    pv = np.concatenate([_chunked(g["ln_gamma"]), _chunked(g["ln_beta"]),
                         _chunked(g["gate_b1"]), _chunked(g["gate_W2"][0])],
                        axis=1)
    import ml_dtypes
    bf = ml_dtypes.bfloat16
    common = {
        "w1t": np.ascontiguousarray(g["gate_W1"].T).astype(bf),
        "pv": pv,
        "sc": np.array([[g["gate_b2"][0] + g["base_logit"][0]]], np.float32),
        "bt": np.ascontiguousarray(
            np.concatenate([g["fast_B"].T, g["deep_B"].T], axis=0)),
    }
    if mode == "lowrank":
        common["caug"] = np.ascontiguousarray(
            np.concatenate([g["fast_C"], g["deep_C"],
                            np.ones((D, 1), np.float32)], axis=1))
    else:
        common["ct"] = np.ascontiguousarray(
            np.concatenate([g["fast_C"].T, g["deep_C"].T], axis=0))
    if has_d:
        common["dv"] = np.ascontiguousarray(
            np.concatenate([_chunked(g["fast_D"] - g["deep_D"]),
                            _chunked(g["deep_D"])], axis=1))

    in_maps = []
    for i in range(NCORES):
        m = dict(common)
        m["kt"] = np.ascontiguousarray(
            k[i * NSH:(i + 1) * NSH, :].T).astype(bf)
        in_maps.append(m)
    return mode, has_d, in_maps


def get_nc(mode, has_d, repeat=1, sim_safe=False):
    key = (mode, has_d, repeat, sim_safe)
    if key not in _NC_CACHE:
        _NC_CACHE[key] = build_nc(mode, has_d, repeat, sim_safe)
    return _NC_CACHE[key]


def assemble(results):
    out = np.empty((N, D), np.float32)
    for i in range(NCORES):
        out[i * NSH:(i + 1) * NSH, :] = results[i]["outT"].astype(np.float32).T
    return out


def kernel(**inputs) -> np.ndarray:
    from concourse.bass_utils import run_bass_kernel_spmd

    mode, has_d, in_maps = prepare(inputs)
    nc = get_nc(mode, has_d)
    res = run_bass_kernel_spmd(nc, in_maps, core_ids=list(range(NCORES)))
    return assemble(res.results)

